# revision 31
# baseline (speedup 1.0000x reference)
"""BiMamba block Trainium2 kernel (v2).

Sharding: 8 cores = (batch 2) x (direction 2) x (d_inner half 2).
Each core runs an identical SPMD program; per-core weights/data encode
(b, dir, h). Host does the final linear gather-sum (+ proj bias + residual).

Per-core layout is channel-major: activations live as (feature, time) tiles so
all matmuls chain without transposes, the causal conv is free-dim shifts, and
the SSM scan runs along the free dim via tensor_tensor_scan.

v2 engine plan (from trace analysis: DVE scan = 2.5ns/elem and is the pacing
engine; gpsimd mult/add = 2.1ns/elem; ACT = 0.9ns/elem; PE 512-free matmul
~0.5us):
  - DVE: the 24 scans, dBu broadcast-mult, LN stats/apply, chunk edge-stitch.
  - GpSimd: C-mult (h*C, big ops only), wc = delta*u, y3 gating mult.
  - ACT: all PSUM->SBUF copies + silu + softplus + the 16 per-block dA exps.
  - PE: all GEMMs at 512 free (two 256-chunks batched), conv as diag matmuls
    (bias folded into the ACT silu), the C-contraction n-tree as 16
    identity-accumulate matmuls + diag(D)@u, transposes.
  - LayerNorm gamma folded into in_proj weight columns; beta folded into a
    per-row bias applied at the PSUM->SBUF copy. Output DMA'd from PSUM.
"""

import numpy as np
import ml_dtypes

import concourse.bass as bass
import concourse.bacc as bacc
import concourse.tile as tile
from concourse import mybir
from concourse.bass_utils import run_bass_kernel_spmd
from concourse.masks import make_identity
from concourse import hw_specs as _hw_specs

# Pin activation tables: keep only the exp+ln set and the silu set populated
# so the act-table-load inserter can never thrash between sets.
_KEEP_TABLES = {"natural_log_exp_and_others", "silu_and_others"}
_orig_gat = _hw_specs.get_activation_tables


def _gat(arch):
    d = _orig_gat(arch)
    return {k: (v if k in _KEEP_TABLES else set()) for k, v in d.items()}


_hw_specs.get_activation_tables = _gat
bacc.get_activation_tables = _gat

F32 = mybir.dt.float32
BF16 = mybir.dt.bfloat16
AL = mybir.AluOpType
AF = mybir.ActivationFunctionType

D_MODEL = 768
D_STATE = 16
D_CONV = 4
D_INNER = 1536
DT_RANK = 48
B_SZ = 2
SEQ = 1024

HALF = 768           # d_inner half per core
TC = 256             # SSM time chunk
TP = 512             # GEMM phase time chunk (pair of SSM chunks)
NP = SEQ // TP       # pairs
NTT = TP // 128      # 128-row time tiles per pair

# packed per-partition f32 column offsets in `cols` (128, NCOL)
CB0 = 0              # conv_b (12)
DTB0 = 12            # dt bias (6)
DP0 = 18             # D param (6)
AA0 = 24             # A = -exp(A_log) (6 blocks x 16) -> cols 24..119
BI0 = 120            # in_proj bias0 = W @ norm_b (18)
EPS0 = 138           # layernorm eps
NCOL = 140

_compiled = {}


def _col(cols, i):
    return cols[:, i:i + 1]


def _bcast_n(ap2d, n):
    """(128, T) AP -> (128, n, T) view with stride-0 n dim."""
    return bass.AP(tensor=ap2d.tensor, offset=ap2d.offset,
                   ap=[ap2d.ap[0], [0, n], ap2d.ap[1]])


def _build_nc(a_chain):
    nc = bacc.Bacc("TRN2", target_bir_lowering=False, num_devices=8)

    xin_d = nc.dram_tensor("xin", [SEQ, D_MODEL], F32, kind="ExternalInput")
    w_iz_d = nc.dram_tensor("w_iz", [128, 6, 2304], BF16, kind="ExternalInput")
    w_xp_d = nc.dram_tensor("w_xp", [128, 12, 112], BF16, kind="ExternalInput")
    w_dt_d = nc.dram_tensor("w_dt", [48, 768], BF16, kind="ExternalInput")
    w_f_d = nc.dram_tensor("w_f", [128, 6, 768], BF16, kind="ExternalInput")
    cols_d = nc.dram_tensor("cols", [128, NCOL], F32, kind="ExternalInput")
    diag_d = nc.dram_tensor("diag", [128, 48, 128], BF16, kind="ExternalInput")
    out_d = nc.dram_tensor("out", [128, 6, SEQ], BF16, kind="ExternalOutput")

    from contextlib import ExitStack
    with ExitStack() as ctx:
        tc = ctx.enter_context(tile.TileContext(nc))
        wp = ctx.enter_context(tc.tile_pool(name="wp", bufs=1))
        lnp = ctx.enter_context(tc.tile_pool(name="lnp", bufs=1))
        lsp = ctx.enter_context(tc.tile_pool(name="lsp", bufs=2))
        xnp = ctx.enter_context(tc.tile_pool(name="xnp", bufs=1))
        xip = ctx.enter_context(tc.tile_pool(name="xip", bufs=2))
        szp = ctx.enter_context(tc.tile_pool(name="szp", bufs=1))
        xsp = ctx.enter_context(tc.tile_pool(name="xsp", bufs=1))
        bcp = ctx.enter_context(tc.tile_pool(name="bcp", bufs=1))
        dlp = ctx.enter_context(tc.tile_pool(name="dlp", bufs=1))
        repp = ctx.enter_context(tc.tile_pool(name="rep", bufs=1))
        crp = ctx.enter_context(tc.tile_pool(name="crp", bufs=1))
        hp = ctx.enter_context(tc.tile_pool(name="hp", bufs=3))
        dbp = ctx.enter_context(tc.tile_pool(name="dbp", bufs=3))
        scn = ctx.enter_context(tc.tile_pool(name="scn", bufs=2))
        wcp = ctx.enter_context(tc.tile_pool(name="wcp", bufs=1))
        y2p = ctx.enter_context(tc.tile_pool(name="y2p", bufs=2))
        y3p = ctx.enter_context(tc.tile_pool(name="y3p", bufs=1))
        obp = ctx.enter_context(tc.tile_pool(name="obp", bufs=1))
        drp = ctx.enter_context(tc.tile_pool(name="drp", bufs=2, space="DRAM"))
        pmP = ctx.enter_context(tc.tile_pool(name="pmP", bufs=2, space="PSUM"))
        ptrP = ctx.enter_context(tc.tile_pool(name="ptrP", bufs=2, space="PSUM"))
        pxpP = ctx.enter_context(tc.tile_pool(name="pxpP", bufs=1, space="PSUM"))
        ybP = ctx.enter_context(tc.tile_pool(name="ybP", bufs=2, space="PSUM"))

        w_iz = wp.tile([128, 6, 2304], BF16, tag="w_iz")
        w_xp = wp.tile([128, 12, 112], BF16, tag="w_xp")
        w_dt = wp.tile([48, 768], BF16, tag="w_dt")
        w_f = wp.tile([128, 6, 768], BF16, tag="w_f")
        cols = wp.tile([128, NCOL], F32, tag="cols")
        diag = wp.tile([128, 48, 128], BF16, tag="diag")
        identb = wp.tile([128, 128], BF16, tag="identb")
        hend = wp.tile([128, 6, 16], BF16, tag="hend")
        nc.sync.dma_start(out=cols[:], in_=cols_d[:])
        nc.sync.dma_start(out=w_iz[:], in_=w_iz_d[:])
        make_identity(nc, identb[:])

        def load_weights():
            nc.sync.dma_start(out=diag[:], in_=diag_d[:])
            nc.sync.dma_start(out=w_xp[:], in_=w_xp_d[:])
            nc.sync.dma_start(out=w_dt[:], in_=w_dt_d[:])
            nc.sync.dma_start(out=w_f[:], in_=w_f_d[:])

        # ---- software-pipelined emission ----
        # iter p: [construct+scans(p,h0)] [front(p+1)] [construct+scans(p,h1)]
        #         [posts(p,h0)] [rest(p+1)] [posts(p,h1)] [out(p)]
        # front = LN + transposes + in_proj-xi (overlaps scans of prev pair)
        # rest  = in_proj-z + conv + x_proj + delta + replication + dA-exps(h0)
        S = {}   # per-pair state

        def front(p):
            t0 = p * TP
            xnc = xnp.tile([128, 6, TP], BF16, tag="xnc", name="xnc")
            for tt in range(NTT):
                xt = lnp.tile([128, D_MODEL], F32, tag="xt", name="xt")
                nc.sync.dma_start(out=xt[:], in_=xin_d[t0 + tt * 128:t0 + (tt + 1) * 128, :])
                st = lsp.tile([128, 3, 6], F32, tag="st", name="st")
                for sg in range(3):
                    nc.vector.bn_stats(out=st[:, sg, :], in_=xt[:, sg * 256:(sg + 1) * 256])
                mv = lsp.tile([128, 2], F32, tag="mv", name="mv")
                nc.vector.bn_aggr(out=mv[:], in_=st[:])
                rs = lsp.tile([128, 1], F32, tag="rs", name="rs")
                nc.scalar.activation(rs[:], mv[:, 1:2], AF.Ln, bias=_col(cols, EPS0))
                nc.scalar.activation(rs[:], rs[:], AF.Exp, scale=-0.5)
                xtb = lnp.tile([128, D_MODEL], BF16, tag="xtb", name="xtb")
                nc.vector.tensor_scalar(xtb[:], xt[:], mv[:, 0:1], rs[:], AL.subtract, AL.mult)
                for dk in range(6):
                    pt = ptrP.tile([128, 128], BF16, tag="pt", name="pt")
                    nc.tensor.transpose(pt[:], xtb[:, dk * 128:(dk + 1) * 128], identb[:])
                    nc.scalar.copy(xnc[:, dk, tt * 128:(tt + 1) * 128], pt[:])
            xic = xip.tile([128, 12, TP + 3], BF16, tag="xic", name="xic")
            if p == 0:
                nc.vector.memset(xic[:, :, 0:3], 0.0)
            else:
                nc.vector.tensor_copy(xic[:, :, 0:3], S[p - 1]["xic"][:, :, TP:TP + 3])
            for m in range(12):
                pm = pmP.tile([128, TP], F32, tag="pm", name="pm")
                for k in range(6):
                    nc.tensor.matmul(pm[:], w_iz[:, k, m * 128:(m + 1) * 128],
                                     xnc[:, k, :], start=(k == 0), stop=(k == 5))
                nc.scalar.activation(xic[:, m, 3:3 + TP], pm[:],
                                     AF.Identity, bias=_col(cols, BI0 + m))
            S[p] = dict(xnc=xnc, xic=xic)

        def rest(p):
            sp = S[p]
            xnc, xic = sp["xnc"], sp["xic"]
            xsc = xsp.tile([128, 12, TP], BF16, tag="xsc", name="xsc")
            for m in range(12):
                pc = pmP.tile([128, TP], F32, tag="pm", name="pm")
                for k in range(4):
                    nc.tensor.matmul(pc[:], diag[:, m * 4 + k, :], xic[:, m, k:k + TP],
                                     start=(k == 0), stop=(k == 3))
                nc.scalar.activation(xsc[:, m, :], pc[:], AF.Silu,
                                     bias=_col(cols, CB0 + m))
            pxp = pxpP.tile([112, TP], F32, tag="pxp", name="pxp")
            for k in range(12):
                nc.tensor.matmul(pxp[:], w_xp[:, k, :], xsc[:, k, :],
                                 start=(k == 0), stop=(k == 11))
            bc = bcp.tile([48, TP], BF16, tag="bc", name="bc")
            dts = bcp.tile([48, TP], BF16, tag="dts", name="dts")
            nc.scalar.copy(bc[0:16, :], pxp[0:16, :])
            nc.scalar.copy(bc[32:48, :], pxp[32:48, :])
            nc.scalar.copy(dts[:], pxp[64:112, :])
            dlc = dlp.tile([128, 6, TP], BF16, tag="dlc", name="dlc")
            for m in range(6):
                pd = pmP.tile([128, TP], F32, tag="pm", name="pd")
                nc.tensor.matmul(pd[:], w_dt[:, m * 128:(m + 1) * 128], dts[:],
                                 start=True, stop=True)
                nc.scalar.activation(dlc[:, m, :], pd[:], AF.Exp, bias=_col(cols, DTB0 + m))
                nc.scalar.activation(dlc[:, m, :], dlc[:, m, :], AF.Ln, bias=1.0)
            szc = szp.tile([128, 6, TP], BF16, tag="szc", name="szc")
            for m in range(12, 18):
                pm = pmP.tile([128, TP], F32, tag="pm", name="pm")
                for k in range(6):
                    nc.tensor.matmul(pm[:], w_iz[:, k, m * 128:(m + 1) * 128],
                                     xnc[:, k, :], start=(k == 0), stop=(k == 5))
                nc.scalar.activation(szc[:, m - 12, :], pm[:], AF.Silu,
                                     bias=_col(cols, BI0 + m))
            breps, creps = [], []
            for half in range(2):
                h0 = half * TC
                brep = repp.tile([128, 16, TC], BF16, tag="brep", name="brep")
                crep = crp.tile([128, 16, TC], BF16, tag="crep", name="crep")
                for p0, dst in ((0, brep), (32, crep)):
                    scr = drp.tile([16, TC], BF16, tag="scr", name="scr")
                    nc.sync.dma_start(out=scr[:], in_=bc[p0:p0 + 16, h0:h0 + TC])
                    sv = scr[:]
                    rd = bass.AP(tensor=sv.tensor, offset=sv.offset,
                                 ap=[[0, 128], sv.ap[0], sv.ap[1]])
                    nc.sync.dma_start(out=dst[:], in_=rd)
                breps.append(brep)
                creps.append(crep)
            sp.update(szc=szc, xsc=xsc, dlc=dlc, breps=breps, creps=creps)
            emit_dA(p, 0)

        def emit_dA(p, half):
            # dA tiles + ACT exps for one half
            if ("dAs%d" % half) in S[p]:
                return
            h0 = half * TC
            dlc = S[p]["dlc"]
            dAs = []
            for m in range(6):
                dA = scn.tile([128, 16, TC], BF16, tag="dA", name="dA")
                for n in range(16):
                    nc.scalar.activation(dA[:, n, :], dlc[:, m, h0:h0 + TC],
                                         AF.Exp, scale=_col(cols, AA0 + m * 16 + n))
                dAs.append(dA)
            S[p]["dAs%d" % half] = dAs

        def scans(p, half):
            sp = S[p]
            c = 2 * p + half
            h0 = half * TC
            dlc, xsc = sp["dlc"], sp["xsc"]
            if ("dAs%d" % half) not in sp:
                emit_dA(p, half)
            dAs = sp["dAs%d" % half]
            wcs = []
            for m in range(6):
                wc = wcp.tile([128, TC], BF16, tag="wc%d" % (m % 2),
                              name="wc")
                nc.gpsimd.tensor_tensor(wc[:], dlc[:, m, h0:h0 + TC],
                                        xsc[:, m, h0:h0 + TC], AL.mult)
                wcs.append(wc)
            dBus = []
            hs = []

            def emit_constr(m):
                dA = dAs[m]
                dBu = dbp.tile([128, 16, TC], BF16, tag="dBu", name="dBu")
                nc.vector.tensor_tensor(dBu[:], _bcast_n(wcs[m][:], 16),
                                        sp["breps"][half][:], AL.mult)
                if c > 0:
                    fx = wcp.tile([128, 16], BF16, tag="fx", name="fx")
                    nc.vector.tensor_tensor(fx[:], dA[:, :, 0], hend[:, m, :], AL.mult)
                    nc.vector.tensor_tensor(dBu[:, :, 0], dBu[:, :, 0], fx[:], AL.add)
                nc.vector.memset(dA[:, :, 0], 0.0)
                dBus.append(dBu)

            def emit_scan(m):
                h = hp.tile([128, 16, TC], BF16, tag="h", name="h")
                nc.vector.tensor_tensor_scan(h[:].rearrange("p n t -> p (n t)"),
                                             dAs[m][:].rearrange("p n t -> p (n t)"),
                                             dBus[m][:].rearrange("p n t -> p (n t)"),
                                             0.0, AL.mult, AL.add)
                nc.vector.tensor_copy(hend[:, m, :], h[:, :, TC - 1])
                hs.append(h)

            def emit_cmult(m):
                # ch = h * C, in place over the dead dBu tile (DVE keeps ports)
                nc.vector.tensor_tensor(dBus[m][:].rearrange("p n t -> p (n t)"),
                                        hs[m][:].rearrange("p n t -> p (n t)"),
                                        sp["creps"][half][:].rearrange("p n t -> p (n t)"),
                                        AL.mult)

            emit_constr(0)
            emit_constr(1)
            for m in range(6):
                emit_scan(m)
                if m + 2 < 6:
                    emit_constr(m + 2)
                if m >= 2:
                    emit_cmult(m - 2)
            emit_cmult(4)
            emit_cmult(5)
            sp["hs%d" % half] = hs
            sp["chs%d" % half] = dBus

        def posts(p, half):
            sp = S[p]
            h0 = half * TC
            xsc, szc = sp["xsc"], sp["szc"]
            chs = sp["chs%d" % half]
            if half == 0:
                sp["y3c"] = y3p.tile([128, 6, TP], BF16, tag="y3c", name="y3c")
            y3c = sp["y3c"]
            for m in range(6):
                du = y2p.tile([128, TC], BF16, tag="du", name="du")
                nc.scalar.activation(du[:], xsc[:, m, h0:h0 + TC],
                                     AF.Identity, scale=_col(cols, DP0 + m))
                yb = ybP.tile([128, TC], F32, tag="yb", name="yb")
                nc.tensor.matmul(yb[:], identb[:], du[:], start=True, stop=False)
                ch = chs[m]
                for n in range(16):
                    nc.tensor.matmul(yb[:], identb[:], ch[:, n, :],
                                     start=False, stop=(n == 15))
                y2 = y2p.tile([128, TC], BF16, tag="y2", name="y2")
                nc.scalar.copy(y2[:], yb[:])
                nc.gpsimd.tensor_tensor(y3c[:, m, h0:h0 + TC], y2[:],
                                        szc[:, m, h0:h0 + TC], AL.mult)

        def outp(p):
            t0 = p * TP
            y3c = S[p]["y3c"]
            for m in range(6):
                po = pmP.tile([128, TP], F32, tag="pm", name="po")
                for k in range(6):
                    nc.tensor.matmul(po[:], w_f[:, k, m * 128:(m + 1) * 128],
                                     y3c[:, k, :], start=(k == 0), stop=(k == 5))
                ob = obp.tile([128, TP], BF16, tag="ob", name="ob")
                nc.scalar.copy(ob[:], po[:])
                nc.sync.dma_start(out=out_d[:, m, t0:t0 + TP], in_=ob[:])

        front(0)
        load_weights()
        rest(0)
        for p in range(NP):
            scans(p, 0)
            if p + 1 < NP:
                front(p + 1)
            scans(p, 1)
            posts(p, 0)
            emit_dA(p, 1)
            if p + 1 < NP:
                rest(p + 1)
            posts(p, 1)
            outp(p)
            if p - 1 >= 0:
                S.pop(p - 1, None)

    nc.finalize()
    return nc


def _to_sb(w, nblk):
    """(nblk*128, X) -> (128, nblk, X) partition-major layout."""
    x = w.shape[1]
    return np.ascontiguousarray(w.reshape(nblk, 128, x).transpose(1, 0, 2))


def _cols_vec(v, nblk):
    """(nblk*128,) -> (128, nblk)."""
    return np.ascontiguousarray(v.reshape(nblk, 128).T)


def _bf(a):
    return np.ascontiguousarray(a.astype(ml_dtypes.bfloat16))


def _prep_weight_set(p, proj_w, h, norm_g, norm_b):
    """p: dict of one direction's mamba params; returns per-core DRAM arrays."""
    sl = slice(h * HALF, (h + 1) * HALF)
    # permute d_inner so own half comes first
    perm = np.concatenate([np.arange(h * HALF, (h + 1) * HALF),
                           np.arange((1 - h) * HALF, (2 - h) * HALF)])
    in_w = np.asarray(p["in_w"], np.float32)
    xi_w = in_w[:D_INNER][perm]                       # (1536, 768)
    z_w = in_w[D_INNER:][sl]                          # (768, 768)
    w_izr = np.concatenate([xi_w, z_w], axis=0)       # (2304, 768) rows x d
    bias0 = w_izr @ np.asarray(norm_b, np.float32)    # (2304,)
    w_izg = w_izr * np.asarray(norm_g, np.float32)[None, :]
    w_iz = _bf(_to_sb(np.ascontiguousarray(w_izg.T), 6))  # (128, 6, 2304)

    xp = np.asarray(p["xproj_w"], np.float32)
    xp_pad = np.zeros((112, D_INNER), np.float32)
    xp_pad[0:16] = xp[DT_RANK:DT_RANK + 16]           # B
    xp_pad[32:48] = xp[DT_RANK + 16:DT_RANK + 32]     # C
    xp_pad[64:112] = xp[0:DT_RANK]                    # dt
    w_xp = _bf(_to_sb(np.ascontiguousarray(xp_pad[:, perm].T), 12))  # (128,12,112)

    dt_w = np.asarray(p["dt_w"], np.float32)[sl]      # (768, 48)
    w_dt = _bf(np.ascontiguousarray(dt_w.T))          # (48, 768)

    out_w = np.asarray(p["out_w"], np.float32)        # (768, 1536)
    w_fold = proj_w @ out_w[:, sl]                    # (768dm, 768dy)
    w_f = _bf(_to_sb(np.ascontiguousarray(w_fold.T), 6))  # (128, 6, 768)

    conv_w = np.asarray(p["conv_w"], np.float32)[perm]    # (1536, 4)
    conv_b = np.asarray(p["conv_b"], np.float32)[perm]
    dt_b = np.asarray(p["dt_b"], np.float32)[sl]
    A = -np.exp(np.asarray(p["A_log"], np.float32))[sl]   # (768, 16)
    Dp = np.asarray(p["D"], np.float32)[sl]
    return w_iz, w_xp, w_dt, w_f, conv_w, conv_b, dt_b, A, Dp, bias0


def kernel(**inputs):
    a_all = np.stack([-np.exp(np.asarray(inputs[p + "A_log"], np.float32)) for p in ("f_", "b_")])
    a_chain = bool(np.allclose(a_all, -np.arange(1, 17, dtype=np.float32)[None, None, :],
                               rtol=1e-6, atol=1e-6))
    key = ("nc", a_chain)
    if key not in _compiled:
        _compiled[key] = _build_nc(a_chain)
    nc = _compiled[key]

    x = np.asarray(inputs["x"], np.float32)
    norm_g = np.asarray(inputs["norm_g"], np.float32)
    norm_b = np.asarray(inputs["norm_b"], np.float32)
    proj_w = np.asarray(inputs["proj_w"], np.float32)
    proj_b = np.asarray(inputs["proj_b"], np.float32)

    # 4 distinct weight sets: (dir, h); shared across batch
    wsets = {}
    for d in range(2):
        pref = "f_" if d == 0 else "b_"
        p = {k: inputs[pref + k] for k in
             ("in_w", "conv_w", "conv_b", "xproj_w", "dt_w", "dt_b", "A_log", "D", "out_w")}
        pw_half = proj_w[:, d * D_MODEL:(d + 1) * D_MODEL]
        for h in range(2):
            w_iz, w_xp, w_dt, w_f, conv_w, conv_b, dt_b, A, Dp, bias0 = \
                _prep_weight_set(p, pw_half, h, norm_g, norm_b)
            cols = np.zeros((128, NCOL), np.float32)
            cols[:, CB0:CB0 + 12] = _cols_vec(conv_b, 12)
            cols[:, DTB0:DTB0 + 6] = _cols_vec(dt_b, 6)
            cols[:, DP0:DP0 + 6] = _cols_vec(Dp, 6)
            cols[:, AA0:AA0 + 96] = A.reshape(6, 128, 16).transpose(1, 0, 2).reshape(128, 96)
            cols[:, EPS0] = 1e-5
            cols[:, BI0:BI0 + 18] = _cols_vec(bias0, 18)
            diag = np.zeros((128, 48, 128), ml_dtypes.bfloat16)
            cwp = conv_w.reshape(12, 128, 4)
            for m in range(12):
                for k in range(4):
                    np.fill_diagonal(diag[:, m * 4 + k, :], cwp[m, :, k])
            wsets[(d, h)] = dict(w_iz=w_iz, w_xp=w_xp, w_dt=w_dt, w_f=w_f,
                                 cols=np.ascontiguousarray(cols),
                                 diag=np.ascontiguousarray(diag))

    in_maps = []
    meta = []
    for b in range(2):
        for d in range(2):
            xb = x[b] if d == 0 else x[b, ::-1]
            xb = np.ascontiguousarray(xb)
            for h in range(2):
                im = dict(wsets[(d, h)])
                im["xin"] = xb
                in_maps.append(im)
                meta.append((b, d, h))

    _compiled["last_in_maps"] = in_maps
    res = run_bass_kernel_spmd(nc, in_maps, core_ids=list(range(8)))

    out = np.tile((proj_b[None, :]).astype(np.float32), (B_SZ, SEQ, 1)) + x
    for i, (b, d, h) in enumerate(meta):
        po = np.asarray(res.results[i]["out"], np.float32)  # (128, 6, 1024)
        po = po.transpose(2, 1, 0).reshape(SEQ, D_MODEL)   # (t, dm)
        if d == 1:
            po = po[::-1]
        out[b] += po
    return out
